# revision 14
# baseline (speedup 1.0000x reference)
"""ExpertBank Trainium2 kernel v2: per-expert [LN-affine-folded Linear(512,1024)
-> GELU(erf) -> Linear(1024,512)] for all 16 experts, expert-parallel over 8
cores (2 experts/core).

v2 vs baseline: host precomputes x_hat (LN applied) AND pre-transposes it to
fp16 [128, KH, N] layout, so the device does zero LN work and zero PE
transposes -- the PE runs only the 2048 GEMM matmuls.  Emission interleaves
G1(e0), G1(e1), G2(e0), G2(e1) per block so the scalar-engine GELU drain of
one expert hides under the other expert's GEMM1.
"""
import numpy as np

import concourse.tile as tile
import concourse.mybir as mybir
from concourse import bacc
from concourse.bass import ds
from concourse.bass_utils import run_bass_kernel_spmd

F32 = mybir.dt.float32
FP16 = mybir.dt.float16

B, S, H, F, E = 4, 2048, 512, 1024, 16
N = B * S                 # 8192 tokens
NCORES = 8
E_LOC = E // NCORES       # 2 experts per core
EPS = 1e-5
TBLK = 1024               # tokens per block
NBLK = N // TBLK          # 8
KH = H // 128             # 4 contraction chunks for GEMM1
KF = F // 128             # 8 contraction chunks for GEMM2
MF = F // 128             # 8 output chunks for GEMM1
MT = TBLK // 128          # 8 token subtiles per block

GELU = mybir.ActivationFunctionType.Gelu
ADD = mybir.AluOpType.add

_COMPILED = None


def _build():
    nc = bacc.Bacc("TRN2", debug=False, enable_asserts=False,
                   target_bir_lowering=False)
    warm_d = nc.dram_tensor("warm", [128, 64], FP16, kind="ExternalInput").ap()
    xT_d = nc.dram_tensor("xT", [NBLK, 128, KH, TBLK], FP16,
                          kind="ExternalInput").ap()
    # W1 mf-major: per (e, mf) a 128KB partition-contiguous chunk so the
    # critical first transfers are small (shrinks kernel-start DMA latency)
    w1_d = nc.dram_tensor("w1", [E_LOC, MF, 128, KH, 128], FP16,
                          kind="ExternalInput").ap()
    w2_d = nc.dram_tensor("w2", [E_LOC, KF, 128, H], FP16,
                          kind="ExternalInput").ap()
    b1_d = nc.dram_tensor("b1c", [128, E_LOC * MF], F32,
                          kind="ExternalInput").ap()
    b2_d = nc.dram_tensor("b2bc", [E_LOC, 128, H], F32,
                          kind="ExternalInput").ap()
    out_d = nc.dram_tensor("out", [N, E_LOC, H], F32, kind="ExternalOutput").ap()

    with tile.TileContext(nc) as tc:
        with tc.tile_pool(name="const", bufs=1) as cst, \
             tc.tile_pool(name="io", bufs=1) as io, \
             tc.tile_pool(name="ps", bufs=1, space="PSUM") as ps:

            def emit_x(b, split=1):
                xt = cst.tile([128, KH, TBLK], FP16, name=f"xt{b}")
                for k in range(KH):
                    for q in range(split):
                        w = TBLK // split
                        nc.sync.dma_start(xt[:, k, ds(q * w, w)],
                                          xT_d[b, :, k, ds(q * w, w)])
                return xt

            # PE clock warm-up: the Tensor engine ramps 0.65 -> 1.2 -> 2.4 GHz
            # over ~3us of execution.  Burn that ramp on throwaway matmuls
            # during the otherwise-idle initial DMA wait so the first real
            # matmuls run at full clock.
            warmt = cst.tile_from(warm_d, name="warmt")
            for _ in range(80):
                pmw = ps.tile([128, 512], F32, name="pm1", tag="pm1", bufs=4)
                nc.tensor.matmul(pmw[0:64, 0:64], warmt[:, 0:64], warmt)

            # critical path: the first GEMM1 group (mf=0, hf=0) needs only
            # xt0 hf0 (4x128KB) + W1 e0 mf0 (128KB); emit those 5 first
            xt = [None] * NBLK
            xt[0] = cst.tile([128, KH, TBLK], FP16, name="xt0")
            for k in range(KH):
                nc.sync.dma_start(xt[0][:, k, ds(0, 512)],
                                  xT_d[0, :, k, ds(0, 512)])
            w1t = [[cst.tile_from(w1_d[e, mf], name=f"w1_{e}_{mf}")
                    for mf in range(MF)] for e in range(1)]
            b1t = cst.tile_from(b1_d, name="b1t")
            for k in range(KH):
                nc.sync.dma_start(xt[0][:, k, ds(512, 512)],
                                  xT_d[0, :, k, ds(512, 512)])
            xt[1] = emit_x(1)
            w2t = [[cst.tile_from(w2_d[e, k], name=f"w2_{e}_{k}")
                    for k in range(KF)] for e in range(1)]
            b2t = [cst.tile_from(b2_d[e], name=f"b2_{e}") for e in range(E_LOC)]
            # expert 1 weights
            w1t.append([cst.tile_from(w1_d[1, mf], name=f"w1_1_{mf}")
                        for mf in range(MF)])
            w2t.append([cst.tile_from(w2_d[1, k], name=f"w2_1_{k}")
                        for k in range(KF)])
            for b in range(2, NBLK):
                xt[b] = emit_x(b)

            def emit_g1(b, e):
                """GEMM1 + GELU: hT[mf] = gelu(W1.T @ xT + b1), fp16."""
                hT = [io.tile([128, TBLK], FP16, name="hT", tag="hT", bufs=24)
                      for _ in range(MF)]
                for hf in range(TBLK // 512):
                    for mf in range(MF):
                        pm1 = ps.tile([128, 512], F32, name="pm1", tag="pm1",
                                      bufs=4)
                        for k in range(KH):
                            nc.tensor.matmul(
                                pm1, w1t[e][mf][:, k],
                                xt[b][:, k, ds(hf * 512, 512)],
                                start=(k == 0), stop=(k == KH - 1))
                        nc.scalar.activation(
                            hT[mf][:, ds(hf * 512, 512)], pm1, GELU,
                            bias=b1t[:, e * MF + mf:e * MF + mf + 1],
                            scale=1.0)
                return hT

            def emit_g2(b, e, hT):
                """GEMM2 + b2: out[tok, :] = hT.T @ W2 + b2."""
                tok0 = b * TBLK
                for mt in range(MT):
                    pm2 = ps.tile([128, H], F32, name="pm2", tag="pm2", bufs=4)
                    for k in range(KF):
                        nc.tensor.matmul(pm2, hT[k][:, ds(mt * 128, 128)],
                                         w2t[e][k], start=(k == 0),
                                         stop=(k == KF - 1))
                    o_t = io.tile([128, H], F32, name="o_t", tag="o_t", bufs=8)
                    nc.vector.tensor_tensor(o_t, pm2, b2t[e], ADD)
                    nc.sync.dma_start(out_d[ds(tok0 + mt * 128, 128), e, :], o_t)

            for b in range(NBLK):
                hTs = [emit_g1(b, e) for e in range(E_LOC)]
                for e in range(E_LOC):
                    emit_g2(b, e, hTs[e])
    nc.compile()
    return nc


def _get_compiled():
    global _COMPILED
    if _COMPILED is None:
        _COMPILED = _build()
    return _COMPILED


def _prepare_in_maps(tokens, ln_g, ln_b, W1, b1, W2, b2):
    x = np.ascontiguousarray(np.asarray(tokens, dtype=np.float32).reshape(N, H))
    # LN stats (float64 internally; matches fp32 reference to ~1e-7 rel)
    x64 = x.astype(np.float64)
    mu = x64.mean(axis=1)
    var = np.square(x64 - mu[:, None]).mean(axis=1)
    rstd = 1.0 / np.sqrt(var + EPS)
    x_hat = ((x64 - mu[:, None]) * rstd[:, None]).astype(np.float16)

    # device layout: [NBLK, 128, KH, TBLK]; element (b,p,k,t) = x_hat[b*TBLK+t, k*128+p]
    xT = np.ascontiguousarray(
        x_hat.reshape(NBLK, TBLK, KH, 128).transpose(0, 3, 2, 1))

    # Fold LN affine into W1/b1: (x_hat*g + b) @ W1 + b1 = x_hat @ (g*W1) + (b@W1 + b1)
    W1 = np.asarray(W1, dtype=np.float32)
    W2 = np.asarray(W2, dtype=np.float32)
    ln_g = np.asarray(ln_g, dtype=np.float32)
    ln_b = np.asarray(ln_b, dtype=np.float32)
    b1 = np.asarray(b1, dtype=np.float32)
    b2 = np.asarray(b2, dtype=np.float32)
    W1eff = (ln_g[:, :, None] * W1).astype(np.float16)
    b1eff = (np.einsum('eh,ehf->ef', ln_b.astype(np.float64),
                       W1.astype(np.float64)) + b1).astype(np.float32)
    W2h = W2.astype(np.float16)

    in_maps = []
    for c in range(NCORES):
        e0 = c * E_LOC
        sl = slice(e0, e0 + E_LOC)
        in_maps.append({
            "warm": np.zeros((128, 64), np.float16),
            "xT": xT,
            "w1": np.ascontiguousarray(
                W1eff[sl].reshape(E_LOC, KH, 128, MF, 128)
                .transpose(0, 3, 2, 1, 4)),
            "w2": np.ascontiguousarray(W2h[sl].reshape(E_LOC, KF, 128, H)),
            "b1c": np.ascontiguousarray(
                b1eff[sl].reshape(E_LOC, MF, 128).transpose(2, 0, 1)
                .reshape(128, E_LOC * MF)),
            "b2bc": np.ascontiguousarray(
                np.broadcast_to(b2[sl][:, None, :], (E_LOC, 128, H))),
        })
    return in_maps


def _run(in_maps, trace=False, **kw):
    nc = _get_compiled()
    return run_bass_kernel_spmd(nc, in_maps, core_ids=list(range(NCORES)),
                                trace=trace, **kw)


def kernel(tokens, ln_g, ln_b, W1, b1, W2, b2):
    in_maps = _prepare_in_maps(tokens, ln_g, ln_b, W1, b1, W2, b2)
    res = _run(in_maps)
    parts = [res.results[c]["out"] for c in range(NCORES)]   # [N, E_LOC, H] each
    full = np.concatenate(parts, axis=1).reshape(B, S, E, H)
    return full.astype(np.float32)
